# revision 5
# baseline (speedup 1.0000x reference)
"""DeepSeekV3 router (moe_routing) Bass kernel for 8x TRN2 NeuronCores.

Data-parallel over tokens (T sharded 8 ways); kernel_DE/bias_E replicated.

z = x@W via main term + fp8 DoubleRow correction:
  main:  x_r (fp16, scaled 2^12) @ W_r (fp16)              [256 cyc/chunk]
  cross: x_r8 @ W_e8 + x_e8 @ W_r8 in ONE fp8e4m3 DoubleRow
         matmul (2 k-tiles per pass, 0.5 cyc/row)          [128 cyc/chunk]
  transpose: one fp32 PE transpose per chunk               [256 cyc/chunk]
Scales: x_e8 stores 2^12*x_e, W_e8 stores 2^20*W_e, W_r8 stores 2^8*W_r,
so both cross products carry 2^20 and z = zm*2^-12 + zc*2^-20, folded into
one stt + sigmoid(scale=2^-12). x/W split at 11 bits (fp16) with 5-bit fp8
residuals -> z error ~2^-16 (measured metric 8.8e-3 vs 2e-2 gate).
"""

import numpy as np

import concourse.bass as bass
import concourse.mybir as mybir
from concourse import bacc
from concourse.bass_utils import run_bass_kernel_spmd
from concourse.masks import make_identity
from concourse.tile import TileContext

F32 = mybir.dt.float32
F32R = mybir.dt.float32r
FP8 = mybir.dt.float8e4
I32 = mybir.dt.int32
U32 = mybir.dt.uint32

T, D, E = 16384, 7168, 256
N_CORES = 8
TOP_K = 8
N_GROUPS = 8
TOPK_GROUPS = 4
EPG = E // N_GROUPS
SCALE = 2.5

P = 128
TS = T // N_CORES
KC = D // P                # 56 contraction chunks
TG = 8                     # chunks per transpose/matmul group
NG = KC // TG              # 7 groups per tile
QC = 14                    # chunks per x-DMA quarter (1792 cols, 7KB descs)
NQ = KC // QC              # 4 quarters
MM_LAG = 3                 # matmul groups lag transposes by this many steps

S_XE = 4096.0              # 2^12: x_e tile scale (and main-term scale)
S_WE = float(2**20)        # W_e8 scale
S_WR = float(2**8)         # W_r8 scale
COMB = float(2**-8)        # zc * COMB + zm == 2^12 * z
UNSC = float(2**-12)       # sigmoid input scale


def build(ts: int = TS) -> bass.Bass:
    nt = ts // P
    nc = bacc.Bacc("TRN2", target_bir_lowering=False)

    x_dram = nc.dram_tensor("x", [ts, D], F32, kind="ExternalInput")
    # host pre-tiles and pre-splits W (see prep_w): w_r = fp32r-rounded W in
    # [P, KC, E]; wx8 = fp8e4 pair [W_e8 | W_r8] in [P, KC, 2, E]
    wr_dram = nc.dram_tensor("w_r", [P, KC, E], mybir.dt.float16, kind="ExternalInput")
    wx8_dram = nc.dram_tensor("wx8", [P, KC, 2, E], FP8, kind="ExternalInput")
    b_dram = nc.dram_tensor("bias", [E], F32, kind="ExternalInput")
    ow_dram = nc.dram_tensor("out_w", [ts, TOP_K], F32, kind="ExternalOutput")
    oi_dram = nc.dram_tensor("out_i", [ts, TOP_K], I32, kind="ExternalOutput")

    with TileContext(nc) as tc:
        with (
            tc.tile_pool(name="consts", bufs=1) as cp,
            tc.tile_pool(name="natp", bufs=6) as natp,
            tc.tile_pool(name="xrp", bufs=4) as xrp,
            tc.tile_pool(name="x8p", bufs=4) as x8p,
            tc.tile_pool(name="stg", bufs=3, space=bass.MemorySpace.PSUM) as stgp,
            tc.tile_pool(name="zm", bufs=1, space=bass.MemorySpace.PSUM) as zmp,
            tc.tile_pool(name="zc", bufs=1, space=bass.MemorySpace.PSUM) as zcp,
            tc.tile_pool(name="rt", bufs=2) as rp,
            tc.tile_pool(name="outp", bufs=3) as op_,
        ):
            # ---- x quarters: [P, 1792] tiles, 7KB per-partition descriptors --
            nat_tiles: dict[tuple, object] = {}

            def load_quarter(i, q):
                natq = natp.tile([P, QC * P], F32, tag="natq", name="natq")
                nat_tiles[(i, q)] = natq
                nc.sync.dma_start(
                    out=natq,
                    in_=x_dram[i * P : (i + 1) * P, q * QC * P : (q + 1) * QC * P],
                )

            # tile 0 quarter 0 arrives as two halves: transposes start sooner
            nat0 = []
            for h in range(2):
                t = natp.tile([P, 7 * P], F32, tag=f"nat0{h}", name=f"nat0{h}")
                nat0.append(t)
                nc.sync.dma_start(
                    out=t, in_=x_dram[0:P, h * 7 * P : (h + 1) * 7 * P]
                )
            for q in range(1, NQ):
                load_quarter(0, q)

            # ---- constants ----
            ident = cp.tile([P, P], F32)
            make_identity(nc, ident)

            # ---- resident weights (host pre-split, byte-copied in) ----
            # one tile per group: independent DMA semaphores, no WAW chain
            w_r_g = []
            wx8_g = []
            for g in range(NG):
                wi = g * TG
                wrt = cp.tile([P, TG, E], mybir.dt.float16, name=f"wr{g}")
                nc.scalar.dma_start(out=wrt, in_=wr_dram[:, wi : wi + TG, :])
                w_r_g.append(wrt)
                wxt = cp.tile([P, TG, 2, E], FP8, name=f"wx{g}")
                nc.scalar.dma_start(out=wxt, in_=wx8_dram[:, wi : wi + TG, :, :])
                wx8_g.append(wxt)

            bias_rep = cp.tile([P, E], F32)
            nc.gpsimd.dma_start(
                out=bias_rep,
                in_=bass.AP(tensor=b_dram, offset=0, ap=[[0, P], [1, E]]),
            )

            iota_i = cp.tile([P, E], I32)
            nc.gpsimd.iota(iota_i, pattern=[[1, E]], base=0, channel_multiplier=0)
            iota_f = cp.tile([P, E], F32)
            nc.vector.tensor_copy(iota_f, iota_i)

            xr_tiles: dict[tuple, object] = {}
            x8_tiles: dict[tuple, object] = {}
            zm_tiles: dict[int, object] = {}
            zc_tiles: dict[int, object] = {}

            stage_tiles: dict[tuple, object] = {}

            def transpose_pe(i, g):
                stage = stgp.tile([P, TG * P], F32, tag="stage")
                stage_tiles[(i, g)] = stage
                for j in range(TG):
                    c = g * TG + j
                    if i == 0 and c < 14:
                        natq, qo = nat0[c // 7], c % 7
                    else:
                        natq, qo = nat_tiles[(i, c // QC)], c % QC
                    nc.tensor.transpose(
                        stage[:, j * P : (j + 1) * P],
                        natq[:, qo * P : (qo + 1) * P],
                        ident,
                    )
                if g == NG - 1:
                    for q in range(NQ):
                        nat_tiles.pop((i, q), None)

            def xops(i, g):
                stage = stage_tiles.pop((i, g))
                # x_r (fp32r, scaled 2^12) -- ACT reads PSUM
                xr = xrp.tile([P, TG * P], mybir.dt.float16, tag="xr", name="xr")
                nc.scalar.activation(
                    xr, stage, mybir.ActivationFunctionType.Copy, scale=S_XE
                )
                # x8: k-tile 0 = fp8(x), k-tile 1 = fp8(2^12 * x_e)
                x8 = x8p.tile([P, 2, TG * P], FP8, tag="x8", name="x8")
                if g % 3 != 2:
                    nc.scalar.copy(x8[:, 0, :], stage)
                else:
                    nc.vector.tensor_copy(x8[:, 0, :], stage)
                nc.vector.scalar_tensor_tensor(
                    x8[:, 1, :],
                    stage,
                    S_XE,
                    xr,
                    op0=mybir.AluOpType.mult,
                    op1=mybir.AluOpType.subtract,
                )
                xr_tiles[(i, g)] = xr
                x8_tiles[(i, g)] = x8

            def matmul_group(i, g):
                xr = xr_tiles.pop((i, g))
                x8 = x8_tiles.pop((i, g))
                if i not in zm_tiles:
                    zm_tiles[i] = zmp.tile([P, 512], F32, tag="zm", name="zm")
                    zc_tiles[i] = zcp.tile([P, 512], F32, tag="zc", name="zc")
                zm = zm_tiles[i]
                zc = zc_tiles[i]
                x8v = x8.rearrange("p two (j t) -> p j two t", t=P)
                for j in range(TG):
                    c = g * TG + j
                    nc.tensor.matmul(
                        zm[:, 0:E],
                        xr[:, j * P : (j + 1) * P],
                        w_r_g[g][:, j, :],
                        start=(c == 0),
                        stop=(c == KC - 1),
                    )
                for j in range(TG):
                    c = g * TG + j
                    nc.tensor.matmul(
                        zc[:, 0:E],
                        x8v[:, j, :, :],
                        wx8_g[g][:, j, :, :],
                        start=(c == 0),
                        stop=(c == KC - 1),
                        perf_mode=mybir.MatmulPerfMode.DoubleRow,
                    )

            evac_tiles: dict[int, tuple] = {}

            def routing_evac(i):
                # first thing after the tile's last matmul: evacuate both
                # PSUM banks so the next tile's accumulation can start
                zm = zm_tiles.pop(i)
                zc = zc_tiles.pop(i)
                zce = rp.tile([P, E], F32, tag="zce")
                nc.scalar.copy(zce, zc[:, 0:E])
                zme = rp.tile([P, E], F32, tag="zme")
                nc.vector.tensor_copy(zme, zm[:, 0:E])
                evac_tiles[i] = (zce, zme)

            def routing(i):
                zce, zme = evac_tiles.pop(i)
                # t = 2^12*z = zm + 2^-8*zc
                tz = rp.tile([P, E], F32, tag="tz")
                nc.vector.scalar_tensor_tensor(
                    tz,
                    zce,
                    COMB,
                    zme,
                    op0=mybir.AluOpType.mult,
                    op1=mybir.AluOpType.add,
                )
                scores = rp.tile([P, E], F32, tag="scores")
                nc.scalar.activation(
                    scores, tz, mybir.ActivationFunctionType.Sigmoid, scale=UNSC
                )

                biased = rp.tile([P, E], F32, tag="biased")
                eng = nc.vector if i == nt - 1 else nc.gpsimd
                eng.tensor_add(biased, scores, bias_rep)

                gmax = rp.tile([P, N_GROUPS * 8], F32, tag="gmax")
                for g in range(N_GROUPS):
                    nc.vector.max(
                        gmax[:, g * 8 : (g + 1) * 8],
                        biased[:, g * EPG : (g + 1) * EPG],
                    )
                gm3 = gmax.rearrange("p (g k) -> p g k", k=8)
                gsc = rp.tile([P, N_GROUPS], F32, tag="gsc")
                gsc3 = gsc.rearrange("p (g k) -> p g k", k=1)
                nc.vector.tensor_add(gsc3, gm3[:, :, 0:1], gm3[:, :, 1:2])

                g8 = rp.tile([P, 8], F32, tag="g8")
                nc.vector.max(g8, gsc)
                maskg = rp.tile([P, N_GROUPS], F32, tag="maskg")
                nc.vector.tensor_scalar(
                    maskg,
                    gsc,
                    g8[:, TOPK_GROUPS - 1 : TOPK_GROUPS],
                    None,
                    op0=mybir.AluOpType.is_ge,
                )

                masked = rp.tile([P, E], F32, tag="masked")
                mg3 = maskg.rearrange("p (g k) -> p g k", k=1)
                eng.tensor_tensor(
                    masked.rearrange("p (g e) -> p g e", g=N_GROUPS),
                    biased.rearrange("p (g e) -> p g e", g=N_GROUPS),
                    mg3.to_broadcast([P, N_GROUPS, EPG]),
                    op=mybir.AluOpType.mult,
                )

                top8 = rp.tile([P, 8], F32, tag="top8")
                nc.vector.max(top8, masked)
                idx = rp.tile([P, 8], U32, tag="idx")
                nc.vector.max_index(idx, top8, masked)
                idxf = rp.tile([P, 8], F32, tag="idxf")
                nc.vector.tensor_copy(idxf, idx)

                wg = rp.tile([P, 8], F32, tag="wg")
                scratch = rp.tile([P, E], F32, tag="scratch")
                for k in range(TOP_K):
                    nc.vector.scalar_tensor_tensor(
                        scratch,
                        iota_f,
                        idxf[:, k : k + 1],
                        scores,
                        op0=mybir.AluOpType.is_equal,
                        op1=mybir.AluOpType.mult,
                        accum_out=wg[:, k : k + 1],
                    )

                ssum = rp.tile([P, 1], F32, tag="ssum")
                nc.vector.tensor_reduce(
                    ssum, wg, axis=mybir.AxisListType.X, op=mybir.AluOpType.add
                )
                nc.vector.tensor_scalar_add(ssum, ssum, 1e-20)
                rinv = rp.tile([P, 1], F32, tag="rinv")
                nc.vector.reciprocal(rinv, ssum)
                nc.vector.tensor_scalar_mul(rinv, rinv, SCALE)

                wout = op_.tile([P, TOP_K], F32, tag="wout")
                nc.vector.tensor_tensor(
                    wout, wg, rinv.to_broadcast([P, TOP_K]), op=mybir.AluOpType.mult
                )
                iout = op_.tile([P, TOP_K], I32, tag="iout")
                nc.vector.tensor_copy(iout, idx)

                nc.scalar.dma_start(out=ow_dram[i * P : (i + 1) * P, :], in_=wout)
                nc.scalar.dma_start(out=oi_dram[i * P : (i + 1) * P, :], in_=iout)

            # flat (tile, group) step stream; matmuls lag transposes by MM_LAG
            steps = [(i, g) for i in range(nt) for g in range(NG)]
            for s, (i, g) in enumerate(steps):
                if i + 1 < nt and 2 <= g < 2 + NQ:
                    load_quarter(i + 1, g - 2)
                transpose_pe(i, g)
                done = None
                if s >= MM_LAG:
                    mi, mg = steps[s - MM_LAG]
                    matmul_group(mi, mg)
                    if mg == NG - 1:
                        routing_evac(mi)
                        done = mi
                xops(i, g)
                if done is not None:
                    routing(done)
            for s in range(len(steps) - MM_LAG, len(steps)):
                mi, mg = steps[s]
                matmul_group(mi, mg)
                if mg == NG - 1:
                    routing_evac(mi)
                    routing(mi)

    nc.compile()
    return nc


def _fp32r(a: np.ndarray) -> np.ndarray:
    """Round fp32 to the 12-bit-significand fp32r grid (round to nearest)."""
    u = np.asarray(a, np.float32).view(np.uint32).astype(np.uint64)
    r = ((u + (1 << 11)) >> 12 << 12) & 0xFFFFFFFF
    return r.astype(np.uint32).view(np.float32)


def _fp8(a: np.ndarray) -> np.ndarray:
    import ml_dtypes

    return np.asarray(a, np.float32).astype(ml_dtypes.float8_e4m3fn)


def prep_w(kernel_DE: np.ndarray) -> dict[str, np.ndarray]:
    """Host-side W split: retile [D, E] -> [P, KC, E] (8KB-contiguous
    partition lines) and pre-compute the fp32r/fp8 operand tensors."""
    w = np.ascontiguousarray(kernel_DE, dtype=np.float32)
    w = np.ascontiguousarray(w.reshape(KC, P, E).transpose(1, 0, 2))
    import ml_dtypes

    w_r16 = w.astype(np.float16)          # 11-bit significand split
    w_r = w_r16.astype(np.float32)
    w_e = w - w_r
    wx8 = np.empty((P, KC, 2, E), dtype=ml_dtypes.float8_e4m3)
    wx8[:, :, 0, :] = _fp8(w_e * S_WE).view(ml_dtypes.float8_e4m3)  # W_e8 2^20
    wx8[:, :, 1, :] = _fp8(w_r * S_WR).view(ml_dtypes.float8_e4m3)  # W_r8 2^8
    return {"w_r": w_r16, "wx8": wx8}


def kernel(x_TD: np.ndarray, kernel_DE: np.ndarray, bias_E: np.ndarray):
    nc = build(TS)
    x_TD = np.ascontiguousarray(x_TD, dtype=np.float32)
    wmap = prep_w(kernel_DE)
    bias_E = np.ascontiguousarray(bias_E, dtype=np.float32)
    in_maps = [
        {
            "x": x_TD[c * TS : (c + 1) * TS],
            "bias": bias_E,
            **wmap,
        }
        for c in range(N_CORES)
    ]
    res = run_bass_kernel_spmd(nc, in_maps, list(range(N_CORES)))
    w = np.concatenate([r["out_w"] for r in res.results], axis=0)
    i = np.concatenate([r["out_i"] for r in res.results], axis=0)
    return w.astype(np.float32), i.astype(np.int32)
